# revision 4
# baseline (speedup 1.0000x reference)
"""VQ codebook lookup kernel for Trainium2, data-parallel over 8 NeuronCores.

Problem: z_e (256, 32768) f32, embeddings (8192, 256) f32.
  dist[n,k] = ||z_n||^2 - 2 z_n.e_k + ||e_k||^2
  indices = argmin_k dist          (int32, shape (32768,))
  z_q = embeddings[indices].T      (f32,  shape (256, 32768))

Sharding: z_e split along N (tokens) across 8 cores; codebook replicated.

Per-core algorithm (N_s = 4096 tokens):
  argmin_k dist[n,k] == argmax_k score[n,k],  score = z.e - 0.5*||e||^2
  - embT (256, 8192) resident in SBUF (two 128-partition halves); z_e shard
    arrives in the exact lhsT layout the PE wants (contraction D on partitions).
  - per 128-token tile: 16 x (128,512) PSUM chunks via 2 accumulating fp32
    matmuls; fused DVE tensor_tensor_reduce adds -0.5*e_sq (replicated in
    SBUF), writes the score strip to SBUF and emits the per-chunk max.
  - max8 over the 16 chunk-maxes -> global max; max_index over the strip ->
    argmax index (exact fp32, matches the reference within fp32 noise).
  - indirect-DMA gather of the winning embedding rows, PE transpose to
    (d, token) orientation, DMA out.
"""

import os
import sys

sys.path.insert(0, "/opt/trn_rl_repo")

import numpy as np

D = 256
N = 32768
K = 8192
NCORES = 8
NS = N // NCORES          # tokens per core
TOK = 128                 # token tile
NTILES = NS // TOK
CHUNK = 512               # K chunk (one PSUM bank of fp32)
NCHUNK = K // CHUNK

_prog_cache = {}


def _build_program(ns_tokens):
    import concourse.bass as bass
    import concourse.bacc as bacc
    import concourse.mybir as mybir
    import concourse.tile as tile
    from concourse.masks import make_identity

    ntiles = ns_tokens // TOK
    f32 = mybir.dt.float32
    u32 = mybir.dt.uint32

    nc = bacc.Bacc("TRN2", target_bir_lowering=False, debug=False,
                   enable_asserts=False, num_devices=NCORES)

    z_dram = nc.dram_tensor("z", [D, ns_tokens], f32, kind="ExternalInput").ap()
    embT_dram = nc.dram_tensor("embT", [D, K], f32, kind="ExternalInput").ap()
    emb_dram = nc.dram_tensor("emb", [K, D], f32, kind="ExternalInput").ap()
    esq_dram = nc.dram_tensor("esq", [128, K], f32, kind="ExternalInput").ap()
    zq_dram = nc.dram_tensor("z_q", [D, ns_tokens], f32, kind="ExternalOutput").ap()
    idx_dram = nc.dram_tensor("idx", [ns_tokens], u32,
                              kind="ExternalOutput").ap()

    with tile.TileContext(nc) as tc:
        with (
            tc.tile_pool(name="const", bufs=1) as const_pool,
            tc.tile_pool(name="zin", bufs=4) as z_pool,
            tc.tile_pool(name="strip", bufs=2) as strip_pool,
            tc.tile_pool(name="small", bufs=4) as small_pool,
            tc.tile_pool(name="gath", bufs=2) as gath_pool,
            tc.tile_pool(name="zqout", bufs=4) as zq_pool,
            tc.tile_pool(name="psum", bufs=4, space="PSUM") as psum_pool,
            tc.tile_pool(name="tpsum", bufs=2, space="PSUM") as tpsum_pool,
        ):
            embT0 = const_pool.tile([128, K], f32, tag="embT0")
            embT1 = const_pool.tile([128, K], f32, tag="embT1")
            esq = const_pool.tile([128, K], f32, tag="esq")
            ident = const_pool.tile([128, 128], f32, tag="ident")
            idx_all = const_pool.tile([128, ntiles], u32, tag="idx_all")

            nc.sync.dma_start(embT0[:], embT_dram[0:128, :])
            nc.sync.dma_start(embT1[:], embT_dram[128:256, :])
            nc.sync.dma_start(esq[:], esq_dram[:])
            make_identity(nc, ident[:])

            for j in range(ntiles):
                js = bass.ts(j, TOK)
                z0 = z_pool.tile([128, TOK], f32, tag="z")
                z1 = z_pool.tile([128, TOK], f32, tag="z")
                nc.sync.dma_start(z0[:], z_dram[0:128, js])
                nc.sync.dma_start(z1[:], z_dram[128:256, js])

                strip = strip_pool.tile([128, K], f32, tag="strip")

                for c in range(NCHUNK):
                    cs = bass.ts(c, CHUNK)
                    ps = psum_pool.tile([128, CHUNK], f32, tag="ps")
                    nc.tensor.matmul(ps[:], z0[:], embT0[:, cs],
                                     start=True, stop=False)
                    nc.tensor.matmul(ps[:], z1[:], embT1[:, cs],
                                     start=False, stop=True)
                    nc.vector.tensor_add(strip[:, cs], ps[:], esq[:, cs])

                m8 = small_pool.tile([128, 8], f32, tag="m8")
                i8 = small_pool.tile([128, 8], u32, tag="i8")
                nc.vector.max(m8[:], strip[:])
                nc.vector.max_index(i8[:], m8[:], strip[:])
                nc.vector.tensor_copy(idx_all[:, j:j + 1], i8[:, 0:1])

                g = gath_pool.tile([128, D], f32, tag="g")
                nc.gpsimd.indirect_dma_start(
                    out=g[:], out_offset=None, in_=emb_dram[:],
                    in_offset=bass.IndirectOffsetOnAxis(ap=i8[:, 0:1], axis=0),
                )
                for h in range(2):
                    tp = tpsum_pool.tile([128, 128], f32, tag="tp")
                    nc.tensor.transpose(tp[:], g[:, bass.ts(h, 128)], ident[:])
                    zq = zq_pool.tile([128, 128], f32, tag="zq")
                    nc.scalar.copy(zq[:], tp[:])
                    nc.sync.dma_start(zq_dram[bass.ts(h, 128), js], zq[:])

            idx_view = idx_dram.rearrange("(t p) -> p t", p=128)
            nc.sync.dma_start(idx_view, idx_all[:])

    nc.compile()
    return nc


def _get_program(ns_tokens):
    if ns_tokens not in _prog_cache:
        _prog_cache[ns_tokens] = _build_program(ns_tokens)
    return _prog_cache[ns_tokens]


def _make_in_maps(z_e, embeddings, ncores):
    z_e = np.ascontiguousarray(z_e, dtype=np.float32)
    emb = np.ascontiguousarray(embeddings, dtype=np.float32)
    embT = np.ascontiguousarray(emb.T)
    esq = -0.5 * np.sum(emb.astype(np.float32) * emb, axis=1, dtype=np.float32)
    esq_rep = np.ascontiguousarray(
        np.broadcast_to(esq[None, :], (128, emb.shape[0])), dtype=np.float32)
    ns = z_e.shape[1] // ncores
    in_maps = []
    for i in range(ncores):
        in_maps.append({
            "z": np.ascontiguousarray(z_e[:, i * ns:(i + 1) * ns]),
            "embT": embT,
            "emb": emb,
            "esq": esq_rep,
        })
    return in_maps


def kernel(z_e, embeddings, _trace=False, _tmpdir=None):
    from concourse import bass_utils

    nc = _get_program(NS)
    in_maps = _make_in_maps(z_e, embeddings, NCORES)
    res = bass_utils.run_bass_kernel_spmd(
        nc, in_maps, core_ids=list(range(NCORES)), trace=_trace, tmpdir=_tmpdir)
    z_q = np.concatenate([np.asarray(r["z_q"]) for r in res.results], axis=1)
    idx = np.concatenate([np.asarray(r["idx"]) for r in res.results])
    if _trace:
        kernel._last_results = res
    return z_q.astype(np.float32, copy=False), idx.astype(np.int32, copy=False)


# revision 5
# speedup vs baseline: 1.5039x; 1.5039x over previous
"""VQ codebook lookup kernel for Trainium2, data-parallel over 8 NeuronCores.

Problem: z_e (256, 32768) f32, embeddings (8192, 256) f32.
  dist[n,k] = ||z_n||^2 - 2 z_n.e_k + ||e_k||^2
  indices = argmin_k dist          (int32, shape (32768,))
  z_q = embeddings[indices].T      (f32,  shape (256, 32768))

Sharding: z_e split along N (tokens) across 8 cores; codebook replicated.

Per-core algorithm (N_s = 4096 tokens):
  argmin_k dist[n,k] == argmax_k score[n,k],  score = z.e - 0.5*||e||^2
  The score matmul runs at full PE rate via an fp16 hi/lo split
  (z = zh + zl, e = eh + el; zh.eh + zh.el + zl.eh reproduces the fp32
  product to ~4e-5 absolute — the dropped zl.el term is ~2^-24; measured
  min top-2 distance gap in this problem is 4.8e-4, so the argmax matches
  full fp32).  -0.5*||e||^2 is folded into the same PSUM accumulation as
  three extra fp16 contraction rows (hi/lo/lolo split of e_sq) against a
  ones vector, so neither DVE nor ACT ever touches a bias add.

  Per 128-token tile: 16 x (128,512) PSUM chunks of 7 accumulating fp16
  matmuls; ACT copies PSUM->SBUF score strip; DVE max8 + max_index give
  the exact argmax over K=8192; indirect-DMA gathers the winning
  embedding rows; PE transposes them to (d, token); DMA out.
"""

import os
import sys

sys.path.insert(0, "/opt/trn_rl_repo")

import numpy as np

D = 256
N = 32768
K = 8192
NCORES = 8
NS = N // NCORES          # tokens per core
TOK = 128                 # token tile
CHUNK = 512               # K chunk (one PSUM bank of fp32)
NCHUNK = K // CHUNK

_prog_cache = {}


def _build_program(ns_tokens):
    import concourse.bass as bass
    import concourse.bacc as bacc
    import concourse.mybir as mybir
    import concourse.tile as tile
    from concourse.masks import make_identity

    ntiles = ns_tokens // TOK
    f32 = mybir.dt.float32
    f16 = mybir.dt.float16
    u32 = mybir.dt.uint32

    nc = bacc.Bacc("TRN2", target_bir_lowering=False, debug=False,
                   enable_asserts=False, num_devices=NCORES)

    zh_dram = nc.dram_tensor("zh", [D, ns_tokens], f16, kind="ExternalInput").ap()
    zl_dram = nc.dram_tensor("zl", [D, ns_tokens], f16, kind="ExternalInput").ap()
    eh_dram = nc.dram_tensor("eh", [D, K], f16, kind="ExternalInput").ap()
    el_dram = nc.dram_tensor("el", [D, K], f16, kind="ExternalInput").ap()
    esq_dram = nc.dram_tensor("esq3", [3, K], f16, kind="ExternalInput").ap()
    emb_dram = nc.dram_tensor("emb", [K, D], f32, kind="ExternalInput").ap()
    zq_dram = nc.dram_tensor("z_q", [D, ns_tokens], f32, kind="ExternalOutput").ap()
    idx_dram = nc.dram_tensor("idx", [ns_tokens], u32, kind="ExternalOutput").ap()

    with tile.TileContext(nc) as tc:
        with (
            tc.tile_pool(name="const", bufs=1) as const_pool,
            tc.tile_pool(name="strip", bufs=2) as strip_pool,
            tc.tile_pool(name="small", bufs=4) as small_pool,
            tc.tile_pool(name="gath", bufs=2) as gath_pool,
            tc.tile_pool(name="zqout", bufs=4) as zq_pool,
            tc.tile_pool(name="psum", bufs=6, space="PSUM") as psum_pool,
            tc.tile_pool(name="tpsum", bufs=2, space="PSUM") as tpsum_pool,
        ):
            eh0 = const_pool.tile([128, K], f16, tag="eh0")
            eh1 = const_pool.tile([128, K], f16, tag="eh1")
            el0 = const_pool.tile([128, K], f16, tag="el0")
            el1 = const_pool.tile([128, K], f16, tag="el1")
            esq3 = const_pool.tile([3, K], f16, tag="esq3")
            ones3 = const_pool.tile([3, 128], f16, tag="ones3")
            ident = const_pool.tile([128, 128], f32, tag="ident")
            idx_all = const_pool.tile([128, ntiles], u32, tag="idx_all")
            zh0a = const_pool.tile([128, ns_tokens], f16, tag="zh0a")
            zh1a = const_pool.tile([128, ns_tokens], f16, tag="zh1a")
            zl0a = const_pool.tile([128, ns_tokens], f16, tag="zl0a")
            zl1a = const_pool.tile([128, ns_tokens], f16, tag="zl1a")

            nc.sync.dma_start(eh0[:], eh_dram[0:128, :])
            nc.sync.dma_start(eh1[:], eh_dram[128:256, :])
            nc.sync.dma_start(el0[:], el_dram[0:128, :])
            nc.sync.dma_start(el1[:], el_dram[128:256, :])
            nc.sync.dma_start(esq3[:], esq_dram[:])
            nc.sync.dma_start(zh0a[:], zh_dram[0:128, :])
            nc.sync.dma_start(zh1a[:], zh_dram[128:256, :])
            nc.sync.dma_start(zl0a[:], zl_dram[0:128, :])
            nc.sync.dma_start(zl1a[:], zl_dram[128:256, :])
            nc.vector.memset(ones3[:], 1.0)
            make_identity(nc, ident[:])

            for j in range(ntiles):
                js = bass.ts(j, TOK)
                strip = strip_pool.tile([128, K], f32, tag="strip")

                for c in range(NCHUNK):
                    cs = bass.ts(c, CHUNK)
                    ps = psum_pool.tile([128, CHUNK], f32, tag="ps")
                    nc.tensor.matmul(ps[:], zh0a[:, js], eh0[:, cs],
                                     start=True, stop=False)
                    nc.tensor.matmul(ps[:], zh1a[:, js], eh1[:, cs],
                                     start=False, stop=False)
                    nc.tensor.matmul(ps[:], zl0a[:, js], eh0[:, cs],
                                     start=False, stop=False)
                    nc.tensor.matmul(ps[:], zl1a[:, js], eh1[:, cs],
                                     start=False, stop=False)
                    nc.tensor.matmul(ps[:], zh0a[:, js], el0[:, cs],
                                     start=False, stop=False)
                    nc.tensor.matmul(ps[:], zh1a[:, js], el1[:, cs],
                                     start=False, stop=False)
                    nc.tensor.matmul(ps[:], ones3[:], esq3[:, cs],
                                     start=False, stop=True)
                    nc.scalar.copy(strip[:, cs], ps[:])

                m8 = small_pool.tile([128, 8], f32, tag="m8")
                i8 = small_pool.tile([128, 8], u32, tag="i8")
                nc.vector.max(m8[:], strip[:])
                nc.vector.max_index(i8[:], m8[:], strip[:])
                nc.vector.tensor_copy(idx_all[:, j:j + 1], i8[:, 0:1])

                g = gath_pool.tile([128, D], f32, tag="g")
                nc.gpsimd.indirect_dma_start(
                    out=g[:], out_offset=None, in_=emb_dram[:],
                    in_offset=bass.IndirectOffsetOnAxis(ap=i8[:, 0:1], axis=0),
                )
                for h in range(2):
                    tp = tpsum_pool.tile([128, 128], f32, tag="tp")
                    nc.tensor.transpose(tp[:], g[:, bass.ts(h, 128)], ident[:])
                    zq = zq_pool.tile([128, 128], f32, tag="zq")
                    nc.scalar.copy(zq[:], tp[:])
                    nc.sync.dma_start(zq_dram[bass.ts(h, 128), js], zq[:])

            idx_view = idx_dram.rearrange("(t p) -> p t", p=128)
            nc.sync.dma_start(idx_view, idx_all[:])

    nc.compile()
    return nc


def _get_program(ns_tokens):
    if ns_tokens not in _prog_cache:
        _prog_cache[ns_tokens] = _build_program(ns_tokens)
    return _prog_cache[ns_tokens]


def _split16(a):
    hi = a.astype(np.float16)
    lo = (a - hi.astype(np.float32)).astype(np.float16)
    return hi, lo


def _make_in_maps(z_e, embeddings, ncores):
    z_e = np.ascontiguousarray(z_e, dtype=np.float32)
    emb = np.ascontiguousarray(embeddings, dtype=np.float32)
    embT = np.ascontiguousarray(emb.T)
    eh, el = _split16(embT)
    zh, zl = _split16(z_e)
    esq = -0.5 * np.sum(emb * emb, axis=1, dtype=np.float32)
    e_h = esq.astype(np.float16)
    r = esq - e_h.astype(np.float32)
    e_l = r.astype(np.float16)
    e_ll = (r - e_l.astype(np.float32)).astype(np.float16)
    esq3 = np.ascontiguousarray(np.stack([e_h, e_l, e_ll], axis=0))
    ns = z_e.shape[1] // ncores
    in_maps = []
    for i in range(ncores):
        sl = slice(i * ns, (i + 1) * ns)
        in_maps.append({
            "zh": np.ascontiguousarray(zh[:, sl]),
            "zl": np.ascontiguousarray(zl[:, sl]),
            "eh": eh,
            "el": el,
            "esq3": esq3,
            "emb": emb,
        })
    return in_maps


def kernel(z_e, embeddings, _trace=False, _tmpdir=None):
    from concourse import bass_utils

    nc = _get_program(NS)
    in_maps = _make_in_maps(z_e, embeddings, NCORES)
    res = bass_utils.run_bass_kernel_spmd(
        nc, in_maps, core_ids=list(range(NCORES)), trace=_trace, tmpdir=_tmpdir)
    z_q = np.concatenate([np.asarray(r["z_q"]) for r in res.results], axis=1)
    idx = np.concatenate([np.asarray(r["idx"]) for r in res.results])
    if _trace:
        kernel._last_results = res
    return z_q.astype(np.float32, copy=False), idx.astype(np.int32, copy=False)


# revision 11
# speedup vs baseline: 1.6098x; 1.0704x over previous
"""VQ codebook lookup kernel for Trainium2, data-parallel over 8 NeuronCores.

Problem: z_e (256, 32768) f32, embeddings (8192, 256) f32.
  dist[n,k] = ||z_n||^2 - 2 z_n.e_k + ||e_k||^2
  indices = argmin_k dist          (int32, shape (32768,))
  z_q = embeddings[indices].T      (f32,  shape (256, 32768))

Sharding: z_e split along N (tokens) across 8 cores; codebook replicated.

Per-core algorithm (N_s = 4096 tokens):
  argmin_k dist[n,k] == argmax_k score[n,k],  score = z.e - 0.5*||e||^2
  The score matmul runs at full PE rate via an fp16 hi/lo split
  (z = zh + zl, e = eh + el; zh.eh + zh.el + zl.eh reproduces the fp32
  product to ~4e-5 absolute — the dropped zl.el term is ~2^-24; measured
  min top-2 distance gap in this problem is 4.8e-4, so the argmax matches
  full fp32).  -0.5*||e||^2 is folded into the same PSUM accumulation as
  three extra fp16 contraction rows (hi/lo/lolo split of e_sq) against a
  ones vector, so neither DVE nor ACT ever touches a bias add.

  Per 128-token tile: 16 x (128,512) PSUM chunks of 7 accumulating fp16
  matmuls; ACT copies PSUM->SBUF score strip; DVE max8 + max_index give
  the exact argmax over K=8192; indirect-DMA gathers the winning
  embedding rows; PE transposes them to (d, token); DMA out.
"""

import os
import sys

sys.path.insert(0, "/opt/trn_rl_repo")

import numpy as np

D = 256
N = 32768
K = 8192
NCORES = 8
NS = N // NCORES          # tokens per core
TOK = 128                 # token tile
CHUNK = 512               # K chunk (one PSUM bank; matmul can't cross banks)
NCHUNK = K // CHUNK
GROUPS = [(0, 6), (6, 6), (12, 4)]  # chunk groups sharing a PSUM residency

_prog_cache = {}


def _build_program(ns_tokens):
    import concourse.bass as bass
    import concourse.bacc as bacc
    import concourse.mybir as mybir
    import concourse.tile as tile
    from concourse.masks import make_identity

    ntiles = ns_tokens // TOK
    f32 = mybir.dt.float32
    f16 = mybir.dt.float16
    u32 = mybir.dt.uint32

    nc = bacc.Bacc("TRN2", target_bir_lowering=False, debug=False,
                   enable_asserts=False, num_devices=NCORES)

    zh_dram = nc.dram_tensor("zh", [D, ns_tokens], f16, kind="ExternalInput").ap()
    zl_dram = nc.dram_tensor("zl", [D, ns_tokens], f16, kind="ExternalInput").ap()
    eh_dram = nc.dram_tensor("eh", [D, K], f16, kind="ExternalInput").ap()
    el_dram = nc.dram_tensor("el", [D, K], f16, kind="ExternalInput").ap()
    esq_dram = nc.dram_tensor("esq3", [3, K], f16, kind="ExternalInput").ap()
    emb_dram = nc.dram_tensor("emb", [K, D], f32, kind="ExternalInput").ap()
    zq_dram = nc.dram_tensor("z_q", [D, ns_tokens], f32, kind="ExternalOutput").ap()
    idx_dram = nc.dram_tensor("idx", [ns_tokens], u32, kind="ExternalOutput").ap()

    with tile.TileContext(nc) as tc:
        with (
            tc.tile_pool(name="const", bufs=1) as const_pool,
            tc.tile_pool(name="strip", bufs=2) as strip_pool,
            tc.tile_pool(name="small", bufs=4) as small_pool,
            tc.tile_pool(name="gath", bufs=2) as gath_pool,
            tc.tile_pool(name="zqout", bufs=4) as zq_pool,
            tc.tile_pool(name="psum", bufs=6, space="PSUM") as psum_pool,
            tc.tile_pool(name="tpsum", bufs=2, space="PSUM") as tpsum_pool,
        ):
            eh0 = const_pool.tile([128, K], f16, tag="eh0")
            eh1 = const_pool.tile([128, K], f16, tag="eh1")
            el0 = const_pool.tile([128, K], f16, tag="el0")
            el1 = const_pool.tile([128, K], f16, tag="el1")
            esq3 = const_pool.tile([3, K], f16, tag="esq3")
            ones3 = const_pool.tile([3, 128], f16, tag="ones3")
            ident = const_pool.tile([128, 128], f32, tag="ident")
            idx_all = const_pool.tile([128, ntiles], u32, tag="idx_all")
            zh0a = const_pool.tile([128, ns_tokens], f16, tag="zh0a")
            zh1a = const_pool.tile([128, ns_tokens], f16, tag="zh1a")
            zl0a = const_pool.tile([128, ns_tokens], f16, tag="zl0a")
            zl1a = const_pool.tile([128, ns_tokens], f16, tag="zl1a")

            nc.sync.dma_start(eh0[:], eh_dram[0:128, :])
            nc.sync.dma_start(eh1[:], eh_dram[128:256, :])
            nc.sync.dma_start(el0[:], el_dram[0:128, :])
            nc.sync.dma_start(el1[:], el_dram[128:256, :])
            nc.sync.dma_start(esq3[:], esq_dram[:])
            nc.sync.dma_start(zh0a[:], zh_dram[0:128, :])
            nc.sync.dma_start(zh1a[:], zh_dram[128:256, :])
            nc.sync.dma_start(zl0a[:], zl_dram[0:128, :])
            nc.sync.dma_start(zl1a[:], zl_dram[128:256, :])
            nc.vector.memset(ones3[:], 1.0)
            make_identity(nc, ident[:])

            for j in range(ntiles):
                js = bass.ts(j, TOK)
                strip = strip_pool.tile([128, K], f32, tag="strip")
                passes = [
                    (zh0a[:, js], eh0), (zh1a[:, js], eh1),
                    (zl0a[:, js], eh0), (zl1a[:, js], eh1),
                    (zh0a[:, js], el0), (zh1a[:, js], el1),
                    (ones3[:], esq3),
                ]
                for g0, cnt in GROUPS:
                    pss = [psum_pool.tile([128, CHUNK], f32, tag="ps",
                                          name=f"ps_{j}_{g0}_{ci}")
                           for ci in range(cnt)]
                    for p, (w, r) in enumerate(passes):
                        for ci in range(cnt):
                            cs = bass.ts(g0 + ci, CHUNK)
                            nc.tensor.matmul(pss[ci][:], w, r[:, cs],
                                             start=(p == 0),
                                             stop=(p == len(passes) - 1))
                    for ci in range(cnt):
                        nc.scalar.copy(strip[:, bass.ts(g0 + ci, CHUNK)],
                                       pss[ci][:])

                m8 = small_pool.tile([128, 8], f32, tag="m8")
                i8 = small_pool.tile([128, 8], u32, tag="i8")
                nc.vector.max(m8[:], strip[:])
                nc.vector.max_index(i8[:], m8[:], strip[:])
                nc.vector.tensor_copy(idx_all[:, j:j + 1], i8[:, 0:1])

                g = gath_pool.tile([128, D], f32, tag="g")
                nc.gpsimd.indirect_dma_start(
                    out=g[:], out_offset=None, in_=emb_dram[:],
                    in_offset=bass.IndirectOffsetOnAxis(ap=i8[:, 0:1], axis=0),
                )
                for h in range(2):
                    tp = tpsum_pool.tile([128, 128], f32, tag="tp")
                    nc.tensor.transpose(tp[:], g[:, bass.ts(h, 128)], ident[:])
                    zq = zq_pool.tile([128, 128], f32, tag="zq")
                    nc.scalar.copy(zq[:], tp[:])
                    nc.sync.dma_start(zq_dram[bass.ts(h, 128), js], zq[:])

            idx_view = idx_dram.rearrange("(t p) -> p t", p=128)
            nc.sync.dma_start(idx_view, idx_all[:])

    nc.compile()
    return nc


def _get_program(ns_tokens):
    if ns_tokens not in _prog_cache:
        _prog_cache[ns_tokens] = _build_program(ns_tokens)
    return _prog_cache[ns_tokens]


def _split16(a):
    hi = a.astype(np.float16)
    lo = (a - hi.astype(np.float32)).astype(np.float16)
    return hi, lo


def _make_in_maps(z_e, embeddings, ncores):
    z_e = np.ascontiguousarray(z_e, dtype=np.float32)
    emb = np.ascontiguousarray(embeddings, dtype=np.float32)
    embT = np.ascontiguousarray(emb.T)
    eh, el = _split16(embT)
    zh, zl = _split16(z_e)
    esq = -0.5 * np.sum(emb * emb, axis=1, dtype=np.float32)
    e_h = esq.astype(np.float16)
    r = esq - e_h.astype(np.float32)
    e_l = r.astype(np.float16)
    e_ll = (r - e_l.astype(np.float32)).astype(np.float16)
    esq3 = np.ascontiguousarray(np.stack([e_h, e_l, e_ll], axis=0))
    ns = z_e.shape[1] // ncores
    in_maps = []
    for i in range(ncores):
        sl = slice(i * ns, (i + 1) * ns)
        in_maps.append({
            "zh": np.ascontiguousarray(zh[:, sl]),
            "zl": np.ascontiguousarray(zl[:, sl]),
            "eh": eh,
            "el": el,
            "esq3": esq3,
            "emb": emb,
        })
    return in_maps


def kernel(z_e, embeddings, _trace=False, _tmpdir=None):
    from concourse import bass_utils

    nc = _get_program(NS)
    in_maps = _make_in_maps(z_e, embeddings, NCORES)
    res = bass_utils.run_bass_kernel_spmd(
        nc, in_maps, core_ids=list(range(NCORES)), trace=_trace, tmpdir=_tmpdir)
    z_q = np.concatenate([np.asarray(r["z_q"]) for r in res.results], axis=1)
    idx = np.concatenate([np.asarray(r["idx"]) for r in res.results])
    if _trace:
        kernel._last_results = res
    return z_q.astype(np.float32, copy=False), idx.astype(np.int32, copy=False)
